# revision 14
# baseline (speedup 1.0000x reference)
"""Multi-head self-attention (causal) for Trainium2, 8 NeuronCores.

Sharding: 8 cores = 4 batches x 2 head-groups (8 heads each).
Each core computes, for its (batch b, head-group g):
    Q^T,K^T = wg @ x_b^T   (feature-major, [512, 2048])
    V       = x_b @ wv^T   (token-major,  [2048, 8, 64+1])  (ones column
                            appended -> softmax denominator rides the PV matmul)
    S^T     = K_h @ Q_h^T  per head   ([keys, queries], causal N-sliced)
    P       = exp(S^T / 8) * tri-mask (no max subtraction; |s/8| <= ~15)
    O^T     = V_aug^T @ P  -> [65, q] per head, row 64 = sum_k P (denominator)
    attn    = O^T[0:64] * broadcast(1/denominator)
    y^T    += wo_g^T-slice @ attn     -> partial [1024, 2048]
Host: y[b] = (yT[b,g=0] + yT[b,g=1])^T.

Projection token-block tb and attention q-tile qt=tb are interleaved so the
PE queue mixes projection matmuls with attention chains (keeps ACT busy and
PE dense). All matmuls run as float32r (TF32-like, same PE rate as bf16).
"""

import numpy as np

import concourse.bass as bass
import concourse.mybir as mybir
import concourse.tile as tile
from concourse import bacc
from concourse.bass_utils import run_bass_kernel_spmd

f32 = mybir.dt.float32
AF = mybir.ActivationFunctionType

MM_DT = mybir.dt.float32r

B = 4
S = 2048
DM = 1024            # d_model
NHG = 2              # head groups (tensor parallel)
F = 512              # features per core (8 heads x 64)
NH = 8               # heads per core
DH = 64
NQT = 4              # q tiles of 512
QW = 512             # q tile width
NKT = 16             # key tiles of 128
NTB = 4              # token blocks in projection phase
KC = 8               # d_model chunks of 128


def build_nc():
    nc = bacc.Bacc(None, target_bir_lowering=False)

    xT_d = nc.dram_tensor("xT", [DM, S], f32, kind="ExternalInput")
    wq_d = nc.dram_tensor("wq", [DM, F], f32, kind="ExternalInput")
    wk_d = nc.dram_tensor("wk", [DM, F], f32, kind="ExternalInput")
    wv_d = nc.dram_tensor("wv", [DM, F], f32, kind="ExternalInput")
    wo_d = nc.dram_tensor("wo", [F, DM], f32, kind="ExternalInput")
    tri_d = nc.dram_tensor("tri", [128, 128], f32, kind="ExternalInput")
    yT_d = nc.dram_tensor("yT", [DM, S], f32, kind="ExternalOutput")

    with tile.TileContext(nc) as tc:
        with (
            tc.tile_pool(name="w", bufs=1) as pool_w,
            tc.tile_pool(name="x", bufs=1) as pool_x,
            tc.tile_pool(name="kq", bufs=1) as pool_kq,
            tc.tile_pool(name="v", bufs=1) as pool_v,
            tc.tile_pool(name="p", bufs=4) as pool_p,
            tc.tile_pool(name="attn", bufs=1) as pool_attn,
            tc.tile_pool(name="misc", bufs=1) as pool_misc,
            tc.tile_pool(name="y", bufs=2) as pool_y,
            tc.tile_pool(name="psa", bufs=2, space="PSUM") as ps_a,
            tc.tile_pool(name="pssc", bufs=2, space="PSUM") as ps_score,
            tc.tile_pool(name="pspv", bufs=2, space="PSUM") as ps_pv,
        ):
            # ---- constants & weights (gpsimd DMA casts fp32 -> f32r) ----
            tri_sb = pool_misc.tile([128, 128], f32, tag="tri")
            nc.sync.dma_start(tri_sb[:], tri_d[:])
            ones_sb = pool_misc.tile([128, 8], f32, tag="ones")
            nc.vector.memset(ones_sb[:], 1.0)

            def load_w(dram, name):
                ts = []
                for kc in range(KC):
                    t = pool_w.tile([128, F], MM_DT, tag=f"w{name}{kc}", name=f"w{name}{kc}")
                    nc.gpsimd.dma_start(t[:], dram[kc * 128:(kc + 1) * 128, :])
                    ts.append(t)
                return ts

            wq_t = load_w(wq_d, "q")
            wk_t = load_w(wk_d, "k")
            wv_t = load_w(wv_d, "v")

            def load_wo():
                # reuses the wq slots (same shape, tag wq{i}) once the last
                # projection released them
                wo_t = []
                for fc in range(4):
                    row = []
                    for oh in range(2):
                        i = fc * 2 + oh
                        t = pool_w.tile([128, 512], MM_DT, tag=f"wq{i}", name=f"wo{fc}{oh}")
                        nc.gpsimd.dma_start(
                            t[:], wo_d[fc * 128:(fc + 1) * 128, oh * 512:(oh + 1) * 512]
                        )
                        row.append(t)
                    wo_t.append(row)
                return wo_t

            # ---- persistent on-chip activations ----
            qT_t = [pool_kq.tile([128, S], MM_DT, tag=f"q{mt}", name=f"qT{mt}") for mt in range(4)]
            kT_t = [pool_kq.tile([128, S], MM_DT, tag=f"k{mt}", name=f"kT{mt}") for mt in range(4)]
            v_t = [pool_v.tile([128, NH, DH + 1], MM_DT, tag=f"v{st}", name=f"v{st}") for st in range(NKT)]

            def project_tb(tb):
                """Q/K/V projections for token block tb (512 tokens)."""
                x_t = []
                for kc in range(KC):
                    t = pool_x.tile([128, 512], MM_DT, tag=f"x{kc}", name=f"x{kc}")
                    nc.gpsimd.dma_start(
                        t[:], xT_d[kc * 128:(kc + 1) * 128, tb * 512:(tb + 1) * 512]
                    )
                    x_t.append(t)

                for W, dstT in ((wq_t, qT_t), (wk_t, kT_t)):
                    for mt in range(4):
                        ps = ps_a.tile([128, 512], f32, tag="qkv", name="psqkv")
                        for kc in range(KC):
                            nc.tensor.matmul(
                                ps[:],
                                W[kc][:, mt * 128:(mt + 1) * 128],
                                x_t[kc][:],
                                start=(kc == 0),
                                stop=(kc == KC - 1),
                            )
                        nc.vector.tensor_copy(
                            dstT[mt][:, tb * 512:(tb + 1) * 512], ps[:]
                        )

                for st in range(4):
                    ps = ps_a.tile([128, 512], f32, tag="qkv", name="psqkv")
                    for kc in range(KC):
                        nc.tensor.matmul(
                            ps[:],
                            x_t[kc][:, st * 128:(st + 1) * 128],
                            wv_t[kc][:],
                            start=(kc == 0),
                            stop=(kc == KC - 1),
                        )
                    vt = v_t[tb * 4 + st]
                    nc.vector.tensor_copy(
                        vt[:, :, 0:DH], ps[:].rearrange("p (h f) -> p h f", h=NH)
                    )
                    nc.vector.tensor_copy(
                        vt[:, :, DH:DH + 1],
                        ones_sb[:].rearrange("p (h o) -> p h o", h=NH),
                    )

            def attend_qt(qt):
                """Attention + output projection for q tile qt (512 queries).

                kt tiles processed in pairs: one [128, 1024] score/P tile
                hosts kts (2i, 2i+1) side by side -> one exp per pair.
                """
                nkt = 4 * qt + 4
                attn_t = [
                    pool_attn.tile([128, QW], MM_DT, tag=f"a{fc}", name=f"attn{fc}")
                    for fc in range(4)
                ]
                for hp in range(4):
                    heads = (2 * hp, 2 * hp + 1)
                    pv = {h: ps_pv.tile([65, QW], f32, tag="pv", name=f"pv{h}") for h in heads}
                    for kp in range(nkt // 2):
                        kts = (2 * kp, 2 * kp + 1)
                        n0s = [max(0, 128 * (kt - 4 * qt)) for kt in kts]
                        for h in heads:
                            ht = h // 2
                            po = (h % 2) * 64
                            sc = ps_score.tile([128, 2 * QW], f32, tag="sc", name="sc")
                            for j, kt in enumerate(kts):
                                nc.tensor.matmul(
                                    sc[:, j * QW + n0s[j]:(j + 1) * QW],
                                    kT_t[ht][po:po + 64, kt * 128:(kt + 1) * 128],
                                    qT_t[ht][po:po + 64, qt * QW + n0s[j]:(qt + 1) * QW],
                                    start=True,
                                    stop=True,
                                )
                            for j, kt in enumerate(kts):
                                d = kt - 4 * qt
                                if d >= 0:
                                    c0 = j * QW + n0s[j]
                                    nc.vector.tensor_tensor(
                                        sc[:, c0:c0 + 128],
                                        sc[:, c0:c0 + 128],
                                        tri_sb[:],
                                        mybir.AluOpType.add,
                                    )
                            p = pool_p.tile([128, 2 * QW], MM_DT, tag="p", name="p")
                            nc.scalar.activation(
                                p[:, n0s[0]:2 * QW], sc[:, n0s[0]:2 * QW],
                                AF.Exp, scale=0.125,
                            )
                            for j, kt in enumerate(kts):
                                nc.tensor.matmul(
                                    pv[h][:, n0s[j]:QW],
                                    v_t[kt][:, h, :],
                                    p[:, j * QW + n0s[j]:(j + 1) * QW],
                                    start=(kt == 0),
                                    stop=(kt == nkt - 1),
                                )
                    for h in heads:
                        ht = h // 2
                        po = (h % 2) * 64
                        drow = pool_misc.tile([1, QW], f32, tag="drow", name="drow")
                        nc.vector.tensor_copy(drow[:], pv[h][64:65, :])
                        srow = pool_misc.tile([1, QW], f32, tag="srow", name="srow")
                        nc.vector.reciprocal_approx_accurate(
                            out=drow[:], in_=drow[:], scratch=srow[:]
                        )
                        dbc = pool_misc.tile([64, QW], f32, tag="dbc", name="dbc")
                        nc.gpsimd.partition_broadcast(dbc[:], drow[:])
                        nc.vector.tensor_tensor(
                            attn_t[ht][po:po + 64, :],
                            pv[h][0:64, :],
                            dbc[:],
                            mybir.AluOpType.mult,
                        )

                for od in range(8):
                    yb = ps_a.tile([128, QW], f32, tag="qkv", name="yb")
                    for fc in range(4):
                        nc.tensor.matmul(
                            yb[:],
                            wo_t[fc][od // 4][:, (od % 4) * 128:(od % 4 + 1) * 128],
                            attn_t[fc][:],
                            start=(fc == 0),
                            stop=(fc == 3),
                        )
                    y_sb = pool_y.tile([128, QW], f32, tag="y", name="ysb")
                    nc.vector.tensor_copy(y_sb[:], yb[:])
                    nc.sync.dma_start(
                        yT_d[od * 128:(od + 1) * 128, qt * QW:(qt + 1) * QW], y_sb[:]
                    )

            for i in range(NTB):
                project_tb(i)
            wo_t = load_wo()
            for i in range(NQT):
                attend_qt(i)

    nc.compile()
    return nc


_NC_CACHE = None


def _ensure_trace_hooks():
    """Dev-only: register the axon NTFF profile hook if the image's antenv
    lacks it, and neuter the artifact upload (no bucket in this container)."""
    import sys
    import types

    import concourse.bass_utils as bu

    bu.upload_artifacts = lambda tmpdir: tmpdir
    try:
        from antenv.axon_hooks import get_axon_ntff_profile_hook  # noqa: F401
        return
    except ImportError:
        pass
    from trn_agent_boot.trn_boot import _ntff_profile_via_ctypes

    hook = _ntff_profile_via_ctypes("/opt/axon/libaxon_pjrt.so")
    mod = types.ModuleType("antenv.axon_hooks")
    mod.get_axon_ntff_profile_hook = lambda: hook

    def set_axon_ntff_profile_hook(h):
        mod.get_axon_ntff_profile_hook = lambda: h

    mod.set_axon_ntff_profile_hook = set_axon_ntff_profile_hook
    sys.modules["antenv.axon_hooks"] = mod


def kernel(x, q_proj, k_proj, v_proj, o_proj, trace=False):
    global _NC_CACHE
    if trace:
        _ensure_trace_hooks()
    x = np.asarray(x, dtype=np.float32)
    q_proj = np.asarray(q_proj, dtype=np.float32)
    k_proj = np.asarray(k_proj, dtype=np.float32)
    v_proj = np.asarray(v_proj, dtype=np.float32)
    o_proj = np.asarray(o_proj, dtype=np.float32)

    tri = np.zeros((128, 128), dtype=np.float32)
    k_idx = np.arange(128)[:, None]
    q_idx = np.arange(128)[None, :]
    tri[k_idx > q_idx] = -1.0e9

    in_maps = []
    for b in range(B):
        xT = np.ascontiguousarray(x[b].T)
        for g in range(NHG):
            rows = slice(g * F, (g + 1) * F)
            in_maps.append(
                {
                    "xT": xT,
                    "wq": np.ascontiguousarray(q_proj[rows, :].T),
                    "wk": np.ascontiguousarray(k_proj[rows, :].T),
                    "wv": np.ascontiguousarray(v_proj[rows, :].T),
                    "wo": np.ascontiguousarray(o_proj[:, rows].T),
                    "tri": tri,
                }
            )

    if _NC_CACHE is None:
        _NC_CACHE = build_nc()
    nc = _NC_CACHE

    res = run_bass_kernel_spmd(nc, in_maps, list(range(2 * B)), trace=trace)

    y = np.empty((B, S, DM), dtype=np.float32)
    for b in range(B):
        acc = res.results[2 * b]["yT"] + res.results[2 * b + 1]["yT"]
        y[b] = acc.T
    if trace:
        return y, res
    return y


# revision 17
# speedup vs baseline: 1.0284x; 1.0284x over previous
"""Multi-head self-attention (causal) for Trainium2, 8 NeuronCores.

Sharding: 8 cores = 4 batches x 2 head-groups (8 heads each).
Each core computes, for its (batch b, head-group g):
    Q^T,K^T = wg @ x_b^T   (feature-major, [512, 2048])
    V       = x_b @ wv^T   (token-major,  [2048, 8, 64+1])  (ones column
                            appended -> softmax denominator rides the PV matmul)
    S^T     = K_h @ Q_h^T  per head   ([keys, queries], causal N-sliced)
    P       = exp(S^T / 8) * tri-mask (no max subtraction; |s/8| <= ~15)
    O^T     = V_aug^T @ P  -> [65, q] per head, row 64 = sum_k P (denominator)
    attn    = O^T[0:64] * broadcast(1/denominator)
    y^T    += wo_g^T-slice @ attn     -> partial [1024, 2048]
Host: y[b] = (yT[b,g=0] + yT[b,g=1])^T.

Projection token-block tb and attention q-tile qt=tb are interleaved so the
PE queue mixes projection matmuls with attention chains (keeps ACT busy and
PE dense). All matmuls run as float32r (TF32-like, same PE rate as bf16).
"""

import numpy as np

import concourse.bass as bass
import concourse.mybir as mybir
import concourse.tile as tile
from concourse import bacc
from concourse.bass_utils import run_bass_kernel_spmd

f32 = mybir.dt.float32
AF = mybir.ActivationFunctionType

MM_DT = mybir.dt.float32r

B = 4
S = 2048
DM = 1024            # d_model
NHG = 2              # head groups (tensor parallel)
F = 512              # features per core (8 heads x 64)
NH = 8               # heads per core
DH = 64
NQT = 4              # q tiles of 512
QW = 512             # q tile width
NKT = 16             # key tiles of 128
NTB = 4              # token blocks in projection phase
KC = 8               # d_model chunks of 128


def build_nc():
    nc = bacc.Bacc(None, target_bir_lowering=False)

    xT_d = nc.dram_tensor("xT", [DM, S], f32, kind="ExternalInput")
    wq_d = nc.dram_tensor("wq", [DM, F], f32, kind="ExternalInput")
    wk_d = nc.dram_tensor("wk", [DM, F], f32, kind="ExternalInput")
    wv_d = nc.dram_tensor("wv", [DM, F], f32, kind="ExternalInput")
    wo_d = nc.dram_tensor("wo", [F, DM], f32, kind="ExternalInput")
    tri_d = nc.dram_tensor("tri", [128, 128], f32, kind="ExternalInput")
    yT_d = nc.dram_tensor("yT", [DM, S], f32, kind="ExternalOutput")

    with tile.TileContext(nc) as tc:
        with (
            tc.tile_pool(name="w", bufs=1) as pool_w,
            tc.tile_pool(name="x", bufs=1) as pool_x,
            tc.tile_pool(name="kq", bufs=1) as pool_kq,
            tc.tile_pool(name="v", bufs=1) as pool_v,
            tc.tile_pool(name="p", bufs=4) as pool_p,
            tc.tile_pool(name="attn", bufs=1) as pool_attn,
            tc.tile_pool(name="misc", bufs=1) as pool_misc,
            tc.tile_pool(name="y", bufs=2) as pool_y,
            tc.tile_pool(name="psa", bufs=2, space="PSUM") as ps_a,
            tc.tile_pool(name="pssc", bufs=2, space="PSUM") as ps_score,
            tc.tile_pool(name="pspv", bufs=2, space="PSUM") as ps_pv,
        ):
            # ---- constants & weights (gpsimd DMA casts fp32 -> f32r) ----
            tri_sb = pool_misc.tile([128, 128], f32, tag="tri")
            nc.sync.dma_start(tri_sb[:], tri_d[:])
            ones_sb = pool_misc.tile([128, 8], f32, tag="ones")
            nc.vector.memset(ones_sb[:], 1.0)

            def load_w(dram, name, cast_dma):
                ts = []
                for kc in range(KC):
                    t = pool_w.tile([128, F], MM_DT, tag=f"w{name}{kc}", name=f"w{name}{kc}")
                    if cast_dma:
                        nc.gpsimd.dma_start(t[:], dram[kc * 128:(kc + 1) * 128, :])
                    else:
                        stg = pool_misc.tile([128, F], f32, tag="wstg", name="wstg", bufs=2)
                        nc.sync.dma_start(stg[:], dram[kc * 128:(kc + 1) * 128, :])
                        nc.vector.tensor_copy(t[:], stg[:])
                    ts.append(t)
                return ts

            wq_t = load_w(wq_d, "q", True)
            wk_t = load_w(wk_d, "k", False)
            wv_t = load_w(wv_d, "v", False)

            def load_wo():
                # reuses the wq slots (same shape, tag wq{i}) once the last
                # projection released them
                wo_t = []
                for fc in range(4):
                    row = []
                    for oh in range(2):
                        i = fc * 2 + oh
                        t = pool_w.tile([128, 512], MM_DT, tag=f"wq{i}", name=f"wo{fc}{oh}")
                        nc.gpsimd.dma_start(
                            t[:], wo_d[fc * 128:(fc + 1) * 128, oh * 512:(oh + 1) * 512]
                        )
                        row.append(t)
                    wo_t.append(row)
                return wo_t

            # ---- persistent on-chip activations ----
            qT_t = [pool_kq.tile([128, S], MM_DT, tag=f"q{mt}", name=f"qT{mt}") for mt in range(4)]
            kT_t = [pool_kq.tile([128, S], MM_DT, tag=f"k{mt}", name=f"kT{mt}") for mt in range(4)]
            v_t = [pool_v.tile([128, NH, DH + 1], MM_DT, tag=f"v{st}", name=f"v{st}") for st in range(NKT)]

            def project_tb(tb):
                """Q/K/V projections for token block tb (512 tokens)."""
                x_t = []
                for kc in range(KC):
                    t = pool_x.tile([128, 512], MM_DT, tag=f"x{kc}", name=f"x{kc}")
                    nc.gpsimd.dma_start(
                        t[:], xT_d[kc * 128:(kc + 1) * 128, tb * 512:(tb + 1) * 512]
                    )
                    x_t.append(t)

                for W, dstT in ((wq_t, qT_t), (wk_t, kT_t)):
                    for mt in range(4):
                        ps = ps_a.tile([128, 512], f32, tag="qkv", name="psqkv")
                        for kc in range(KC):
                            nc.tensor.matmul(
                                ps[:],
                                W[kc][:, mt * 128:(mt + 1) * 128],
                                x_t[kc][:],
                                start=(kc == 0),
                                stop=(kc == KC - 1),
                            )
                        nc.vector.tensor_copy(
                            dstT[mt][:, tb * 512:(tb + 1) * 512], ps[:]
                        )

                for st in range(4):
                    ps = ps_a.tile([128, 512], f32, tag="qkv", name="psqkv")
                    for kc in range(KC):
                        nc.tensor.matmul(
                            ps[:],
                            x_t[kc][:, st * 128:(st + 1) * 128],
                            wv_t[kc][:],
                            start=(kc == 0),
                            stop=(kc == KC - 1),
                        )
                    vt = v_t[tb * 4 + st]
                    nc.vector.tensor_copy(
                        vt[:, :, 0:DH], ps[:].rearrange("p (h f) -> p h f", h=NH)
                    )
                    nc.vector.tensor_copy(
                        vt[:, :, DH:DH + 1],
                        ones_sb[:].rearrange("p (h o) -> p h o", h=NH),
                    )

            def attend_qt(qt):
                """Attention + output projection for q tile qt (512 queries).

                kt tiles processed in pairs: one [128, 1024] score/P tile
                hosts kts (2i, 2i+1) side by side -> one exp per pair.
                """
                nkt = 4 * qt + 4
                attn_t = [
                    pool_attn.tile([128, QW], MM_DT, tag=f"a{fc}", name=f"attn{fc}")
                    for fc in range(4)
                ]
                for hp in range(4):
                    heads = (2 * hp, 2 * hp + 1)
                    pv = {h: ps_pv.tile([65, QW], f32, tag="pv", name=f"pv{h}") for h in heads}
                    for kp in range(nkt // 2):
                        kts = (2 * kp, 2 * kp + 1)
                        n0s = [max(0, 128 * (kt - 4 * qt)) for kt in kts]
                        for h in heads:
                            ht = h // 2
                            po = (h % 2) * 64
                            sc = ps_score.tile([128, 2 * QW], f32, tag="sc", name="sc")
                            for j, kt in enumerate(kts):
                                nc.tensor.matmul(
                                    sc[:, j * QW + n0s[j]:(j + 1) * QW],
                                    kT_t[ht][po:po + 64, kt * 128:(kt + 1) * 128],
                                    qT_t[ht][po:po + 64, qt * QW + n0s[j]:(qt + 1) * QW],
                                    start=True,
                                    stop=True,
                                )
                            for j, kt in enumerate(kts):
                                d = kt - 4 * qt
                                if d >= 0:
                                    c0 = j * QW + n0s[j]
                                    nc.vector.tensor_tensor(
                                        sc[:, c0:c0 + 128],
                                        sc[:, c0:c0 + 128],
                                        tri_sb[:],
                                        mybir.AluOpType.add,
                                    )
                            p = pool_p.tile([128, 2 * QW], MM_DT, tag="p", name="p")
                            nc.scalar.activation(
                                p[:, n0s[0]:2 * QW], sc[:, n0s[0]:2 * QW],
                                AF.Exp, scale=0.125,
                            )
                            for j, kt in enumerate(kts):
                                nc.tensor.matmul(
                                    pv[h][:, n0s[j]:QW],
                                    v_t[kt][:, h, :],
                                    p[:, j * QW + n0s[j]:(j + 1) * QW],
                                    start=(kt == 0),
                                    stop=(kt == nkt - 1),
                                )
                    for h in heads:
                        ht = h // 2
                        po = (h % 2) * 64
                        drow = pool_misc.tile([1, QW], f32, tag="drow", name="drow")
                        nc.vector.tensor_copy(drow[:], pv[h][64:65, :])
                        ocp = pool_misc.tile([64, QW], f32, tag="ocp", name="ocp", bufs=2)
                        nc.vector.tensor_copy(ocp[:], pv[h][0:64, :])
                        srow = pool_misc.tile([1, QW], f32, tag="srow", name="srow")
                        nc.vector.reciprocal_approx_accurate(
                            out=drow[:], in_=drow[:], scratch=srow[:]
                        )
                        dbc = pool_misc.tile([64, QW], f32, tag="dbc", name="dbc")
                        nc.gpsimd.partition_broadcast(dbc[:], drow[:])
                        nc.vector.tensor_tensor(
                            attn_t[ht][po:po + 64, :],
                            ocp[:],
                            dbc[:],
                            mybir.AluOpType.mult,
                        )

                for od in range(8):
                    yb = ps_a.tile([128, QW], f32, tag="qkv", name="yb")
                    for fc in range(4):
                        nc.tensor.matmul(
                            yb[:],
                            wo_t[fc][od // 4][:, (od % 4) * 128:(od % 4 + 1) * 128],
                            attn_t[fc][:],
                            start=(fc == 0),
                            stop=(fc == 3),
                        )
                    y_sb = pool_y.tile([128, QW], f32, tag="y", name="ysb")
                    nc.vector.tensor_copy(y_sb[:], yb[:])
                    nc.sync.dma_start(
                        yT_d[od * 128:(od + 1) * 128, qt * QW:(qt + 1) * QW], y_sb[:]
                    )

            for i in range(NTB):
                project_tb(i)
            wo_t = load_wo()
            for i in range(NQT):
                attend_qt(i)

    nc.compile()
    return nc


_NC_CACHE = None


def _ensure_trace_hooks():
    """Dev-only: register the axon NTFF profile hook if the image's antenv
    lacks it, and neuter the artifact upload (no bucket in this container)."""
    import sys
    import types

    import concourse.bass_utils as bu

    bu.upload_artifacts = lambda tmpdir: tmpdir
    try:
        from antenv.axon_hooks import get_axon_ntff_profile_hook  # noqa: F401
        return
    except ImportError:
        pass
    from trn_agent_boot.trn_boot import _ntff_profile_via_ctypes

    hook = _ntff_profile_via_ctypes("/opt/axon/libaxon_pjrt.so")
    mod = types.ModuleType("antenv.axon_hooks")
    mod.get_axon_ntff_profile_hook = lambda: hook

    def set_axon_ntff_profile_hook(h):
        mod.get_axon_ntff_profile_hook = lambda: h

    mod.set_axon_ntff_profile_hook = set_axon_ntff_profile_hook
    sys.modules["antenv.axon_hooks"] = mod


def kernel(x, q_proj, k_proj, v_proj, o_proj, trace=False):
    global _NC_CACHE
    if trace:
        _ensure_trace_hooks()
    x = np.asarray(x, dtype=np.float32)
    q_proj = np.asarray(q_proj, dtype=np.float32)
    k_proj = np.asarray(k_proj, dtype=np.float32)
    v_proj = np.asarray(v_proj, dtype=np.float32)
    o_proj = np.asarray(o_proj, dtype=np.float32)

    tri = np.zeros((128, 128), dtype=np.float32)
    k_idx = np.arange(128)[:, None]
    q_idx = np.arange(128)[None, :]
    tri[k_idx > q_idx] = -1.0e9

    in_maps = []
    for b in range(B):
        xT = np.ascontiguousarray(x[b].T)
        for g in range(NHG):
            rows = slice(g * F, (g + 1) * F)
            in_maps.append(
                {
                    "xT": xT,
                    "wq": np.ascontiguousarray(q_proj[rows, :].T),
                    "wk": np.ascontiguousarray(k_proj[rows, :].T),
                    "wv": np.ascontiguousarray(v_proj[rows, :].T),
                    "wo": np.ascontiguousarray(o_proj[:, rows].T),
                    "tri": tri,
                }
            )

    if _NC_CACHE is None:
        _NC_CACHE = build_nc()
    nc = _NC_CACHE

    res = run_bass_kernel_spmd(nc, in_maps, list(range(2 * B)), trace=trace)

    y = np.empty((B, S, DM), dtype=np.float32)
    for b in range(B):
        acc = res.results[2 * b]["yT"] + res.results[2 * b + 1]["yT"]
        y[b] = acc.T
    if trace:
        return y, res
    return y


# revision 19
# speedup vs baseline: 1.0471x; 1.0182x over previous
"""Multi-head self-attention (causal) for Trainium2, 8 NeuronCores.

Sharding: 8 cores = 4 batches x 2 head-groups (8 heads each).
Each core computes, for its (batch b, head-group g):
    Q^T,K^T = wg @ x_b^T   (feature-major, [512, 2048])
    V       = x_b @ wv^T   (token-major,  [2048, 8, 64+1])  (ones column
                            appended -> softmax denominator rides the PV matmul)
    S^T     = K_h @ Q_h^T  per head   ([keys, queries], causal N-sliced)
    P       = exp(S^T / 8) * tri-mask (no max subtraction; |s/8| <= ~15)
    O^T     = V_aug^T @ P  -> [65, q] per head, row 64 = sum_k P (denominator)
    attn    = O^T[0:64] * broadcast(1/denominator)
    y^T    += wo_g^T-slice @ attn     -> partial [1024, 2048]
Host: y[b] = (yT[b,g=0] + yT[b,g=1])^T.

Projection token-block tb and attention q-tile qt=tb are interleaved so the
PE queue mixes projection matmuls with attention chains (keeps ACT busy and
PE dense). All matmuls run as float32r (TF32-like, same PE rate as bf16).
"""

import numpy as np

import concourse.bass as bass
import concourse.mybir as mybir
import concourse.tile as tile
from concourse import bacc
from concourse.bass_utils import run_bass_kernel_spmd

f32 = mybir.dt.float32
AF = mybir.ActivationFunctionType

MM_DT = mybir.dt.float32r

B = 4
S = 2048
DM = 1024            # d_model
NHG = 2              # head groups (tensor parallel)
F = 512              # features per core (8 heads x 64)
NH = 8               # heads per core
DH = 64
NQT = 4              # q tiles of 512
QW = 512             # q tile width
NKT = 16             # key tiles of 128
NTB = 4              # token blocks in projection phase
KC = 8               # d_model chunks of 128


def build_nc():
    nc = bacc.Bacc(None, target_bir_lowering=False)

    xT_d = nc.dram_tensor("xT", [DM, S], f32, kind="ExternalInput")
    wq_d = nc.dram_tensor("wq", [DM, F], f32, kind="ExternalInput")
    wk_d = nc.dram_tensor("wk", [DM, F], f32, kind="ExternalInput")
    wv_d = nc.dram_tensor("wv", [DM, F], f32, kind="ExternalInput")
    wo_d = nc.dram_tensor("wo", [F, DM], f32, kind="ExternalInput")
    tri_d = nc.dram_tensor("tri", [128, 128], f32, kind="ExternalInput")
    yT_d = nc.dram_tensor("yT", [DM, S], f32, kind="ExternalOutput")

    with tile.TileContext(nc) as tc:
        with (
            tc.tile_pool(name="w", bufs=1) as pool_w,
            tc.tile_pool(name="x", bufs=1) as pool_x,
            tc.tile_pool(name="kq", bufs=1) as pool_kq,
            tc.tile_pool(name="v", bufs=1) as pool_v,
            tc.tile_pool(name="p", bufs=3) as pool_p,
            tc.tile_pool(name="attn", bufs=1) as pool_attn,
            tc.tile_pool(name="misc", bufs=1) as pool_misc,
            tc.tile_pool(name="y", bufs=2) as pool_y,
            tc.tile_pool(name="psa", bufs=2, space="PSUM") as ps_a,
            tc.tile_pool(name="pssc", bufs=2, space="PSUM") as ps_score,
            tc.tile_pool(name="pspv", bufs=2, space="PSUM") as ps_pv,
        ):
            # ---- constants & weights (gpsimd DMA casts fp32 -> f32r) ----
            tri_sb = pool_misc.tile([128, 128], f32, tag="tri")
            nc.sync.dma_start(tri_sb[:], tri_d[:])
            ones_sb = pool_misc.tile([128, 8], f32, tag="ones")
            nc.vector.memset(ones_sb[:], 1.0)

            def load_w(dram, name, cast_dma):
                ts = []
                for kc in range(KC):
                    t = pool_w.tile([128, F], MM_DT, tag=f"w{name}{kc}", name=f"w{name}{kc}")
                    if cast_dma:
                        nc.gpsimd.dma_start(t[:], dram[kc * 128:(kc + 1) * 128, :])
                    else:
                        stg = pool_misc.tile([128, F], f32, tag="wstg", name="wstg", bufs=2)
                        nc.sync.dma_start(stg[:], dram[kc * 128:(kc + 1) * 128, :])
                        nc.vector.tensor_copy(t[:], stg[:])
                    ts.append(t)
                return ts

            wq_t = load_w(wq_d, "q", True)
            wk_t = load_w(wk_d, "k", False)
            wv_t = load_w(wv_d, "v", False)

            def load_wo():
                # reuses the wq slots (same shape, tag wq{i}) once the last
                # projection released them
                wo_t = []
                for fc in range(4):
                    row = []
                    for oh in range(2):
                        i = fc * 2 + oh
                        t = pool_w.tile([128, 512], MM_DT, tag=f"wq{i}", name=f"wo{fc}{oh}")
                        nc.gpsimd.dma_start(
                            t[:], wo_d[fc * 128:(fc + 1) * 128, oh * 512:(oh + 1) * 512]
                        )
                        row.append(t)
                    wo_t.append(row)
                return wo_t

            # ---- persistent on-chip activations ----
            qT_t = [pool_kq.tile([128, S], MM_DT, tag=f"q{mt}", name=f"qT{mt}") for mt in range(4)]
            kT_t = [pool_kq.tile([128, S], MM_DT, tag=f"k{mt}", name=f"kT{mt}") for mt in range(4)]
            v_t = [pool_v.tile([128, NH, DH + 1], MM_DT, tag=f"v{st}", name=f"v{st}") for st in range(NKT)]

            def project_tb(tb):
                """Q/K/V projections for token block tb (512 tokens)."""
                x_t = []
                for kc in range(KC):
                    t = pool_x.tile([128, 512], MM_DT, tag=f"x{kc}", name=f"x{kc}")
                    nc.gpsimd.dma_start(
                        t[:], xT_d[kc * 128:(kc + 1) * 128, tb * 512:(tb + 1) * 512]
                    )
                    x_t.append(t)

                for W, dstT in ((wq_t, qT_t), (wk_t, kT_t)):
                    for mt in range(4):
                        ps = ps_a.tile([128, 512], f32, tag="qkv", name="psqkv")
                        for kc in range(KC):
                            nc.tensor.matmul(
                                ps[:],
                                W[kc][:, mt * 128:(mt + 1) * 128],
                                x_t[kc][:],
                                start=(kc == 0),
                                stop=(kc == KC - 1),
                            )
                        nc.vector.tensor_copy(
                            dstT[mt][:, tb * 512:(tb + 1) * 512], ps[:]
                        )

                for st in range(4):
                    ps = ps_a.tile([128, 512], f32, tag="qkv", name="psqkv")
                    for kc in range(KC):
                        nc.tensor.matmul(
                            ps[:],
                            x_t[kc][:, st * 128:(st + 1) * 128],
                            wv_t[kc][:],
                            start=(kc == 0),
                            stop=(kc == KC - 1),
                        )
                    vt = v_t[tb * 4 + st]
                    nc.vector.tensor_copy(
                        vt[:, :, 0:DH], ps[:].rearrange("p (h f) -> p h f", h=NH)
                    )
                    nc.vector.tensor_copy(
                        vt[:, :, DH:DH + 1],
                        ones_sb[:].rearrange("p (h o) -> p h o", h=NH),
                    )

            def attend_qt(qt):
                """Attention for q tile qt (512 queries); returns attn tiles.

                kt tiles processed in pairs: one [128, 1024] score/P tile
                hosts kts (2i, 2i+1) side by side -> one exp per pair.
                """
                nkt = 4 * qt + 4
                attn_t = [
                    pool_attn.tile([128, QW], MM_DT, tag=f"a{fc}", name=f"attn{fc}", bufs=2)
                    for fc in range(4)
                ]
                for hp in range(4):
                    heads = (2 * hp, 2 * hp + 1)
                    pv = {h: ps_pv.tile([65, QW], f32, tag="pv", name=f"pv{h}") for h in heads}
                    for kp in range(nkt // 2):
                        kts = (2 * kp, 2 * kp + 1)
                        n0s = [max(0, 128 * (kt - 4 * qt)) for kt in kts]
                        for h in heads:
                            ht = h // 2
                            po = (h % 2) * 64
                            sc = ps_score.tile([128, 2 * QW], f32, tag="sc", name="sc")
                            for j, kt in enumerate(kts):
                                nc.tensor.matmul(
                                    sc[:, j * QW + n0s[j]:(j + 1) * QW],
                                    kT_t[ht][po:po + 64, kt * 128:(kt + 1) * 128],
                                    qT_t[ht][po:po + 64, qt * QW + n0s[j]:(qt + 1) * QW],
                                    start=True,
                                    stop=True,
                                )
                            for j, kt in enumerate(kts):
                                d = kt - 4 * qt
                                if d >= 0:
                                    c0 = j * QW + n0s[j]
                                    nc.vector.tensor_tensor(
                                        sc[:, c0:c0 + 128],
                                        sc[:, c0:c0 + 128],
                                        tri_sb[:],
                                        mybir.AluOpType.add,
                                    )
                            p = pool_p.tile([128, 2 * QW], MM_DT, tag="p", name="p")
                            nc.scalar.activation(
                                p[:, n0s[0]:2 * QW], sc[:, n0s[0]:2 * QW],
                                AF.Exp, scale=0.125,
                            )
                            for j, kt in enumerate(kts):
                                nc.tensor.matmul(
                                    pv[h][:, n0s[j]:QW],
                                    v_t[kt][:, h, :],
                                    p[:, j * QW + n0s[j]:(j + 1) * QW],
                                    start=(kt == 0),
                                    stop=(kt == nkt - 1),
                                )
                    for h in heads:
                        ht = h // 2
                        po = (h % 2) * 64
                        drow = pool_misc.tile([1, QW], f32, tag="drow", name="drow")
                        nc.vector.tensor_copy(drow[:], pv[h][64:65, :])
                        ocp = pool_misc.tile([64, QW], f32, tag="ocp", name="ocp", bufs=2)
                        nc.vector.tensor_copy(ocp[:], pv[h][0:64, :])
                        srow = pool_misc.tile([1, QW], f32, tag="srow", name="srow")
                        nc.vector.reciprocal_approx_accurate(
                            out=drow[:], in_=drow[:], scratch=srow[:]
                        )
                        dbc = pool_misc.tile([64, QW], f32, tag="dbc", name="dbc")
                        nc.gpsimd.partition_broadcast(dbc[:], drow[:])
                        nc.vector.tensor_tensor(
                            attn_t[ht][po:po + 64, :],
                            ocp[:],
                            dbc[:],
                            mybir.AluOpType.mult,
                        )

                return attn_t

            def oproj_qt(qt, attn_t):
                for od in range(8):
                    yb = ps_a.tile([128, QW], f32, tag="qkv", name="yb")
                    for fc in range(4):
                        nc.tensor.matmul(
                            yb[:],
                            wo_t[fc][od // 4][:, (od % 4) * 128:(od % 4 + 1) * 128],
                            attn_t[fc][:],
                            start=(fc == 0),
                            stop=(fc == 3),
                        )
                    y_sb = pool_y.tile([128, QW], f32, tag="y", name="ysb")
                    nc.vector.tensor_copy(y_sb[:], yb[:])
                    nc.sync.dma_start(
                        yT_d[od * 128:(od + 1) * 128, qt * QW:(qt + 1) * QW], y_sb[:]
                    )

            for i in range(NTB):
                project_tb(i)
            wo_t = load_wo()
            prev = None
            for i in range(NQT):
                at = attend_qt(i)
                if prev is not None:
                    oproj_qt(i - 1, prev)
                prev = at
            oproj_qt(NQT - 1, prev)

    nc.compile()
    return nc


_NC_CACHE = None


def _ensure_trace_hooks():
    """Dev-only: register the axon NTFF profile hook if the image's antenv
    lacks it, and neuter the artifact upload (no bucket in this container)."""
    import sys
    import types

    import concourse.bass_utils as bu

    bu.upload_artifacts = lambda tmpdir: tmpdir
    try:
        from antenv.axon_hooks import get_axon_ntff_profile_hook  # noqa: F401
        return
    except ImportError:
        pass
    from trn_agent_boot.trn_boot import _ntff_profile_via_ctypes

    hook = _ntff_profile_via_ctypes("/opt/axon/libaxon_pjrt.so")
    mod = types.ModuleType("antenv.axon_hooks")
    mod.get_axon_ntff_profile_hook = lambda: hook

    def set_axon_ntff_profile_hook(h):
        mod.get_axon_ntff_profile_hook = lambda: h

    mod.set_axon_ntff_profile_hook = set_axon_ntff_profile_hook
    sys.modules["antenv.axon_hooks"] = mod


def kernel(x, q_proj, k_proj, v_proj, o_proj, trace=False):
    global _NC_CACHE
    if trace:
        _ensure_trace_hooks()
    x = np.asarray(x, dtype=np.float32)
    q_proj = np.asarray(q_proj, dtype=np.float32)
    k_proj = np.asarray(k_proj, dtype=np.float32)
    v_proj = np.asarray(v_proj, dtype=np.float32)
    o_proj = np.asarray(o_proj, dtype=np.float32)

    tri = np.zeros((128, 128), dtype=np.float32)
    k_idx = np.arange(128)[:, None]
    q_idx = np.arange(128)[None, :]
    tri[k_idx > q_idx] = -1.0e9

    in_maps = []
    for b in range(B):
        xT = np.ascontiguousarray(x[b].T)
        for g in range(NHG):
            rows = slice(g * F, (g + 1) * F)
            in_maps.append(
                {
                    "xT": xT,
                    "wq": np.ascontiguousarray(q_proj[rows, :].T),
                    "wk": np.ascontiguousarray(k_proj[rows, :].T),
                    "wv": np.ascontiguousarray(v_proj[rows, :].T),
                    "wo": np.ascontiguousarray(o_proj[:, rows].T),
                    "tri": tri,
                }
            )

    if _NC_CACHE is None:
        _NC_CACHE = build_nc()
    nc = _NC_CACHE

    res = run_bass_kernel_spmd(nc, in_maps, list(range(2 * B)), trace=trace)

    y = np.empty((B, S, DM), dtype=np.float32)
    for b in range(B):
        acc = res.results[2 * b]["yT"] + res.results[2 * b + 1]["yT"]
        y[b] = acc.T
    if trace:
        return y, res
    return y


# revision 20
# speedup vs baseline: 1.0613x; 1.0135x over previous
"""Multi-head self-attention (causal) for Trainium2, 8 NeuronCores.

Sharding: 8 cores = 4 batches x 2 head-groups (8 heads each).
Each core computes, for its (batch b, head-group g):
    Q^T,K^T = wg @ x_b^T   (feature-major, [512, 2048])
    V       = x_b @ wv^T   (token-major,  [2048, 8, 64+1])  (ones column
                            appended -> softmax denominator rides the PV matmul)
    S^T     = K_h @ Q_h^T  per head   ([keys, queries], causal N-sliced)
    P       = exp(S^T / 8) * tri-mask (no max subtraction; |s/8| <= ~15)
    O^T     = V_aug^T @ P  -> [65, q] per head, row 64 = sum_k P (denominator)
    attn    = O^T[0:64] * broadcast(1/denominator)
    y^T    += wo_g^T-slice @ attn     -> partial [1024, 2048]
Host: y[b] = (yT[b,g=0] + yT[b,g=1])^T.

Projection token-block tb and attention q-tile qt=tb are interleaved so the
PE queue mixes projection matmuls with attention chains (keeps ACT busy and
PE dense). All matmuls run as float32r (TF32-like, same PE rate as bf16).
"""

import numpy as np

import concourse.bass as bass
import concourse.mybir as mybir
import concourse.tile as tile
from concourse import bacc
from concourse.bass_utils import run_bass_kernel_spmd

f32 = mybir.dt.float32
AF = mybir.ActivationFunctionType

MM_DT = mybir.dt.float32r

B = 4
S = 2048
DM = 1024            # d_model
NHG = 2              # head groups (tensor parallel)
F = 512              # features per core (8 heads x 64)
NH = 8               # heads per core
DH = 64
NQT = 4              # q tiles of 512
QW = 512             # q tile width
NKT = 16             # key tiles of 128
NTB = 4              # token blocks in projection phase
KC = 8               # d_model chunks of 128


def build_nc():
    nc = bacc.Bacc(None, target_bir_lowering=False)

    xT_d = nc.dram_tensor("xT", [DM, S], f32, kind="ExternalInput")
    wq_d = nc.dram_tensor("wq", [DM, F], f32, kind="ExternalInput")
    wk_d = nc.dram_tensor("wk", [DM, F], f32, kind="ExternalInput")
    wv_d = nc.dram_tensor("wv", [DM, F], f32, kind="ExternalInput")
    wo_d = nc.dram_tensor("wo", [F, DM], f32, kind="ExternalInput")
    tri_d = nc.dram_tensor("tri", [128, 128], f32, kind="ExternalInput")
    yT_d = nc.dram_tensor("yT", [DM, S], f32, kind="ExternalOutput")

    with tile.TileContext(nc) as tc:
        with (
            tc.tile_pool(name="w", bufs=1) as pool_w,
            tc.tile_pool(name="x", bufs=1) as pool_x,
            tc.tile_pool(name="kq", bufs=1) as pool_kq,
            tc.tile_pool(name="v", bufs=1) as pool_v,
            tc.tile_pool(name="p", bufs=3) as pool_p,
            tc.tile_pool(name="attn", bufs=1) as pool_attn,
            tc.tile_pool(name="misc", bufs=1) as pool_misc,
            tc.tile_pool(name="y", bufs=2) as pool_y,
            tc.tile_pool(name="psa", bufs=2, space="PSUM") as ps_a,
            tc.tile_pool(name="pssc", bufs=2, space="PSUM") as ps_score,
            tc.tile_pool(name="pspv", bufs=2, space="PSUM") as ps_pv,
        ):
            # ---- constants & weights (gpsimd DMA casts fp32 -> f32r) ----
            tri_sb = pool_misc.tile([128, 128], f32, tag="tri")
            nc.sync.dma_start(tri_sb[:], tri_d[:])
            ones_sb = pool_misc.tile([128, 8], f32, tag="ones")
            nc.vector.memset(ones_sb[:], 1.0)

            def load_w(dram, name, cast_dma):
                ts = []
                for kc in range(KC):
                    t = pool_w.tile([128, F], MM_DT, tag=f"w{name}{kc}", name=f"w{name}{kc}")
                    if cast_dma:
                        nc.gpsimd.dma_start(t[:], dram[kc * 128:(kc + 1) * 128, :])
                    else:
                        stg = pool_misc.tile([128, F], f32, tag="wstg", name="wstg", bufs=2)
                        nc.sync.dma_start(stg[:], dram[kc * 128:(kc + 1) * 128, :])
                        nc.vector.tensor_copy(t[:], stg[:])
                    ts.append(t)
                return ts

            wq_t = load_w(wq_d, "q", True)
            wk_t = load_w(wk_d, "k", False)
            wv_t = load_w(wv_d, "v", False)

            def load_wo():
                # reuses the wq slots (same shape, tag wq{i}) once the last
                # projection released them
                wo_t = []
                for fc in range(4):
                    row = []
                    for oh in range(2):
                        i = fc * 2 + oh
                        t = pool_w.tile([128, 512], MM_DT, tag=f"wq{i}", name=f"wo{fc}{oh}")
                        nc.gpsimd.dma_start(
                            t[:], wo_d[fc * 128:(fc + 1) * 128, oh * 512:(oh + 1) * 512]
                        )
                        row.append(t)
                    wo_t.append(row)
                return wo_t

            # ---- persistent on-chip activations ----
            qT_t = [pool_kq.tile([128, S], MM_DT, tag=f"q{mt}", name=f"qT{mt}") for mt in range(4)]
            kT_t = [pool_kq.tile([128, S], MM_DT, tag=f"k{mt}", name=f"kT{mt}") for mt in range(4)]
            v_t = [pool_v.tile([128, NH, DH + 1], MM_DT, tag=f"v{st}", name=f"v{st}") for st in range(NKT)]

            def project_tb(tb):
                """Q/K/V projections for token block tb (512 tokens)."""
                x_t = []
                for kc in range(KC):
                    t = pool_x.tile([128, 512], MM_DT, tag=f"x{kc}", name=f"x{kc}")
                    nc.gpsimd.dma_start(
                        t[:], xT_d[kc * 128:(kc + 1) * 128, tb * 512:(tb + 1) * 512]
                    )
                    x_t.append(t)

                for W, dstT in ((wq_t, qT_t), (wk_t, kT_t)):
                    for mt in range(4):
                        ps = ps_a.tile([128, 512], f32, tag="qkv", name="psqkv")
                        for kc in range(KC):
                            nc.tensor.matmul(
                                ps[:],
                                W[kc][:, mt * 128:(mt + 1) * 128],
                                x_t[kc][:],
                                start=(kc == 0),
                                stop=(kc == KC - 1),
                            )
                        nc.vector.tensor_copy(
                            dstT[mt][:, tb * 512:(tb + 1) * 512], ps[:]
                        )

                for st in range(4):
                    ps = ps_a.tile([128, 512], f32, tag="qkv", name="psqkv")
                    for kc in range(KC):
                        nc.tensor.matmul(
                            ps[:],
                            x_t[kc][:, st * 128:(st + 1) * 128],
                            wv_t[kc][:],
                            start=(kc == 0),
                            stop=(kc == KC - 1),
                        )
                    vt = v_t[tb * 4 + st]
                    nc.vector.tensor_copy(
                        vt[:, :, 0:DH], ps[:].rearrange("p (h f) -> p h f", h=NH)
                    )
                    nc.vector.tensor_copy(
                        vt[:, :, DH:DH + 1],
                        ones_sb[:].rearrange("p (h o) -> p h o", h=NH),
                    )

            def attend_qt(qt):
                """Attention for q tile qt (512 queries); returns attn tiles.

                kt tiles processed in pairs: one [128, 1024] score/P tile
                hosts kts (2i, 2i+1) side by side -> one exp per pair.
                """
                nkt = 4 * qt + 4
                attn_t = [
                    pool_attn.tile([128, QW], MM_DT, tag=f"a{fc}", name=f"attn{fc}", bufs=2)
                    for fc in range(4)
                ]
                for hp in range(4):
                    heads = (2 * hp, 2 * hp + 1)
                    pv = {h: ps_pv.tile([65, QW], f32, tag="pv", name=f"pv{h}") for h in heads}
                    for kp in range(nkt // 2):
                        kts = (2 * kp, 2 * kp + 1)
                        n0s = [max(0, 128 * (kt - 4 * qt)) for kt in kts]
                        # score matmuls for both heads adjacent: the K=64 MMs
                        # land on disjoint PE row groups (0-63 / 64-127) and
                        # run concurrently
                        scs = {}
                        for j, kt in enumerate(kts):
                            for h in heads:
                                if h not in scs:
                                    scs[h] = ps_score.tile(
                                        [128, 2 * QW], f32, tag="sc", name="sc"
                                    )
                                ht = h // 2
                                po = (h % 2) * 64
                                nc.tensor.matmul(
                                    scs[h][:, j * QW + n0s[j]:(j + 1) * QW],
                                    kT_t[ht][po:po + 64, kt * 128:(kt + 1) * 128],
                                    qT_t[ht][po:po + 64, qt * QW + n0s[j]:(qt + 1) * QW],
                                    start=True,
                                    stop=True,
                                )
                        for h in heads:
                            for j, kt in enumerate(kts):
                                d = kt - 4 * qt
                                if d >= 0:
                                    c0 = j * QW + n0s[j]
                                    nc.vector.tensor_tensor(
                                        scs[h][:, c0:c0 + 128],
                                        scs[h][:, c0:c0 + 128],
                                        tri_sb[:],
                                        mybir.AluOpType.add,
                                    )
                        ps = {}
                        for h in heads:
                            ps[h] = pool_p.tile([128, 2 * QW], MM_DT, tag="p", name="p")
                            nc.scalar.activation(
                                ps[h][:, n0s[0]:2 * QW], scs[h][:, n0s[0]:2 * QW],
                                AF.Exp, scale=0.125,
                            )
                        for h in heads:
                            for j, kt in enumerate(kts):
                                nc.tensor.matmul(
                                    pv[h][:, n0s[j]:QW],
                                    v_t[kt][:, h, :],
                                    ps[h][:, j * QW + n0s[j]:(j + 1) * QW],
                                    start=(kt == 0),
                                    stop=(kt == nkt - 1),
                                )
                    for h in heads:
                        ht = h // 2
                        po = (h % 2) * 64
                        drow = pool_misc.tile([1, QW], f32, tag="drow", name="drow")
                        nc.vector.tensor_copy(drow[:], pv[h][64:65, :])
                        ocp = pool_misc.tile([64, QW], f32, tag="ocp", name="ocp", bufs=2)
                        nc.vector.tensor_copy(ocp[:], pv[h][0:64, :])
                        srow = pool_misc.tile([1, QW], f32, tag="srow", name="srow")
                        nc.vector.reciprocal_approx_accurate(
                            out=drow[:], in_=drow[:], scratch=srow[:]
                        )
                        dbc = pool_misc.tile([64, QW], f32, tag="dbc", name="dbc")
                        nc.gpsimd.partition_broadcast(dbc[:], drow[:])
                        nc.vector.tensor_tensor(
                            attn_t[ht][po:po + 64, :],
                            ocp[:],
                            dbc[:],
                            mybir.AluOpType.mult,
                        )

                return attn_t

            def oproj_qt(qt, attn_t):
                for od in range(8):
                    yb = ps_a.tile([128, QW], f32, tag="qkv", name="yb")
                    for fc in range(4):
                        nc.tensor.matmul(
                            yb[:],
                            wo_t[fc][od // 4][:, (od % 4) * 128:(od % 4 + 1) * 128],
                            attn_t[fc][:],
                            start=(fc == 0),
                            stop=(fc == 3),
                        )
                    y_sb = pool_y.tile([128, QW], f32, tag="y", name="ysb")
                    nc.vector.tensor_copy(y_sb[:], yb[:])
                    nc.sync.dma_start(
                        yT_d[od * 128:(od + 1) * 128, qt * QW:(qt + 1) * QW], y_sb[:]
                    )

            for i in range(NTB):
                project_tb(i)
            wo_t = load_wo()
            prev = None
            for i in range(NQT):
                at = attend_qt(i)
                if prev is not None:
                    oproj_qt(i - 1, prev)
                prev = at
            oproj_qt(NQT - 1, prev)

    nc.compile()
    return nc


_NC_CACHE = None


def _ensure_trace_hooks():
    """Dev-only: register the axon NTFF profile hook if the image's antenv
    lacks it, and neuter the artifact upload (no bucket in this container)."""
    import sys
    import types

    import concourse.bass_utils as bu

    bu.upload_artifacts = lambda tmpdir: tmpdir
    try:
        from antenv.axon_hooks import get_axon_ntff_profile_hook  # noqa: F401
        return
    except ImportError:
        pass
    from trn_agent_boot.trn_boot import _ntff_profile_via_ctypes

    hook = _ntff_profile_via_ctypes("/opt/axon/libaxon_pjrt.so")
    mod = types.ModuleType("antenv.axon_hooks")
    mod.get_axon_ntff_profile_hook = lambda: hook

    def set_axon_ntff_profile_hook(h):
        mod.get_axon_ntff_profile_hook = lambda: h

    mod.set_axon_ntff_profile_hook = set_axon_ntff_profile_hook
    sys.modules["antenv.axon_hooks"] = mod


def kernel(x, q_proj, k_proj, v_proj, o_proj, trace=False):
    global _NC_CACHE
    if trace:
        _ensure_trace_hooks()
    x = np.asarray(x, dtype=np.float32)
    q_proj = np.asarray(q_proj, dtype=np.float32)
    k_proj = np.asarray(k_proj, dtype=np.float32)
    v_proj = np.asarray(v_proj, dtype=np.float32)
    o_proj = np.asarray(o_proj, dtype=np.float32)

    tri = np.zeros((128, 128), dtype=np.float32)
    k_idx = np.arange(128)[:, None]
    q_idx = np.arange(128)[None, :]
    tri[k_idx > q_idx] = -1.0e9

    in_maps = []
    for b in range(B):
        xT = np.ascontiguousarray(x[b].T)
        for g in range(NHG):
            rows = slice(g * F, (g + 1) * F)
            in_maps.append(
                {
                    "xT": xT,
                    "wq": np.ascontiguousarray(q_proj[rows, :].T),
                    "wk": np.ascontiguousarray(k_proj[rows, :].T),
                    "wv": np.ascontiguousarray(v_proj[rows, :].T),
                    "wo": np.ascontiguousarray(o_proj[:, rows].T),
                    "tri": tri,
                }
            )

    if _NC_CACHE is None:
        _NC_CACHE = build_nc()
    nc = _NC_CACHE

    res = run_bass_kernel_spmd(nc, in_maps, list(range(2 * B)), trace=trace)

    y = np.empty((B, S, DM), dtype=np.float32)
    for b in range(B):
        acc = res.results[2 * b]["yT"] + res.results[2 * b + 1]["yT"]
        y[b] = acc.T
    if trace:
        return y, res
    return y


# revision 21
# speedup vs baseline: 1.2002x; 1.1309x over previous
"""Multi-head self-attention (causal) for Trainium2, 8 NeuronCores.

Sharding: 8 cores = 4 batches x 2 head-groups (8 heads each).
Each core computes, for its (batch b, head-group g):
    Q^T,K^T = wg @ x_b^T   (feature-major)
    V       = x_b @ wv^T   (token-major, ones column appended so the softmax
                            denominator rides the PV matmul)
    S^T     = K_h @ Q_h^T  per head   ([keys, queries], causal N-sliced)
    P       = exp(S^T / 8) * tri-mask (no max subtraction; |s/8| <= ~15)
    O^T     = V_aug^T @ P  -> [65, q] per head, row 64 = sum_k P
    attn    = O^T[0:64] * broadcast(1/denominator)
    y^T    += wo_g^T-slice @ attn     -> partial [1024, 2048]
Host: y[b] = (yT[b,g=0] + yT[b,g=1])^T.

Pipelining: the PE executes its queue strictly in order, so each kt-pair's
PV matmuls are emitted one unit later (their exp has then finished) and
projection matmul chains for the NEXT token block are woven between
attention units as PE filler while ACT (exp) is the rate limiter.
All matmuls are float32r (TF32-like, same PE rate as bf16).
"""

import numpy as np

import concourse.bass as bass
import concourse.mybir as mybir
import concourse.tile as tile
from concourse import bacc
from concourse.bass_utils import run_bass_kernel_spmd

f32 = mybir.dt.float32
AF = mybir.ActivationFunctionType

MM_DT = mybir.dt.float32r

B = 4
S = 2048
DM = 1024            # d_model
NHG = 2              # head groups (tensor parallel)
F = 512              # features per core (8 heads x 64)
NH = 8               # heads per core
DH = 64
NQT = 4              # q tiles of 512
QW = 512             # q tile width
NKT = 16             # key tiles of 128
NTB = 4              # token blocks in projection phase
KC = 8               # d_model chunks of 128


def build_nc():
    nc = bacc.Bacc(None, target_bir_lowering=False)

    xT_d = nc.dram_tensor("xT", [DM, S], f32, kind="ExternalInput")
    wq_d = nc.dram_tensor("wq", [DM, F], f32, kind="ExternalInput")
    wk_d = nc.dram_tensor("wk", [DM, F], f32, kind="ExternalInput")
    wv_d = nc.dram_tensor("wv", [DM, F], f32, kind="ExternalInput")
    wo_d = nc.dram_tensor("wo", [F, DM], f32, kind="ExternalInput")
    tri_d = nc.dram_tensor("tri", [128, 128], f32, kind="ExternalInput")
    yT_d = nc.dram_tensor("yT", [DM, S], f32, kind="ExternalOutput")

    with tile.TileContext(nc) as tc:
        with (
            tc.tile_pool(name="w", bufs=1) as pool_w,
            tc.tile_pool(name="x", bufs=1) as pool_x,
            tc.tile_pool(name="kq", bufs=1) as pool_kq,
            tc.tile_pool(name="v", bufs=1) as pool_v,
            tc.tile_pool(name="p", bufs=3) as pool_p,
            tc.tile_pool(name="attn", bufs=1) as pool_attn,
            tc.tile_pool(name="misc", bufs=1) as pool_misc,
            tc.tile_pool(name="y", bufs=2) as pool_y,
            tc.tile_pool(name="psa", bufs=2, space="PSUM") as ps_a,
            tc.tile_pool(name="pssc", bufs=2, space="PSUM") as ps_score,
            tc.tile_pool(name="pspv", bufs=2, space="PSUM") as ps_pv,
        ):
            # ---- constants & weights ----
            tri_sb = pool_misc.tile([128, 128], f32, tag="tri")
            nc.sync.dma_start(tri_sb[:], tri_d[:])
            ones_sb = pool_misc.tile([128, 8], f32, tag="ones")
            nc.vector.memset(ones_sb[:], 1.0)

            def load_w(dram, name, cast_dma):
                ts = []
                for kc in range(KC):
                    t = pool_w.tile([128, F], MM_DT, tag=f"w{name}{kc}", name=f"w{name}{kc}")
                    if cast_dma:
                        nc.gpsimd.dma_start(t[:], dram[kc * 128:(kc + 1) * 128, :])
                    else:
                        stg = pool_misc.tile([128, F], f32, tag="wstg", name="wstg", bufs=2)
                        nc.sync.dma_start(stg[:], dram[kc * 128:(kc + 1) * 128, :])
                        nc.vector.tensor_copy(t[:], stg[:])
                    ts.append(t)
                return ts

            wq_t = load_w(wq_d, "q", True)
            wk_t = load_w(wk_d, "k", False)
            wv_t = load_w(wv_d, "v", False)

            # ---- persistent on-chip activations ----
            kT_t = [pool_kq.tile([128, S], MM_DT, tag=f"k{mt}", name=f"kT{mt}") for mt in range(4)]
            v_t = [pool_v.tile([128, NH, DH + 1], MM_DT, tag=f"v{st}", name=f"v{st}") for st in range(NKT)]

            def load_x(tb):
                x_t = []
                for kc in range(KC):
                    t = pool_x.tile([128, 512], MM_DT, tag=f"x{kc}", name=f"x{kc}")
                    nc.gpsimd.dma_start(
                        t[:], xT_d[kc * 128:(kc + 1) * 128, tb * 512:(tb + 1) * 512]
                    )
                    x_t.append(t)
                return x_t

            def make_proj_chains(tb, x_t):
                """12 closures, each emitting one output tile's 8-MM chain."""
                qcur = [
                    pool_kq.tile([128, 512], MM_DT, tag=f"q{mt}", name=f"q{mt}", bufs=2)
                    for mt in range(4)
                ]
                chains = []

                def q_chain(mt):
                    def go():
                        ps = ps_a.tile([128, 512], f32, tag="qkv", name="psqkv")
                        for kc in range(KC):
                            nc.tensor.matmul(
                                ps[:],
                                wq_t[kc][:, mt * 128:(mt + 1) * 128],
                                x_t[kc][:],
                                start=(kc == 0),
                                stop=(kc == KC - 1),
                            )
                        nc.vector.tensor_copy(qcur[mt][:], ps[:])
                    return go

                def k_chain(mt):
                    def go():
                        ps = ps_a.tile([128, 512], f32, tag="qkv", name="psqkv")
                        for kc in range(KC):
                            nc.tensor.matmul(
                                ps[:],
                                wk_t[kc][:, mt * 128:(mt + 1) * 128],
                                x_t[kc][:],
                                start=(kc == 0),
                                stop=(kc == KC - 1),
                            )
                        nc.vector.tensor_copy(
                            kT_t[mt][:, tb * 512:(tb + 1) * 512], ps[:]
                        )
                    return go

                def v_chain(st):
                    def go():
                        ps = ps_a.tile([128, 512], f32, tag="qkv", name="psqkv")
                        for kc in range(KC):
                            nc.tensor.matmul(
                                ps[:],
                                x_t[kc][:, st * 128:(st + 1) * 128],
                                wv_t[kc][:],
                                start=(kc == 0),
                                stop=(kc == KC - 1),
                            )
                        vt = v_t[tb * 4 + st]
                        nc.vector.tensor_copy(
                            vt[:, :, 0:DH], ps[:].rearrange("p (h f) -> p h f", h=NH)
                        )
                        nc.vector.tensor_copy(
                            vt[:, :, DH:DH + 1],
                            ones_sb[:].rearrange("p (h o) -> p h o", h=NH),
                        )
                    return go

                for mt in range(4):
                    chains.append(q_chain(mt))
                for mt in range(4):
                    chains.append(k_chain(mt))
                for st in range(4):
                    chains.append(v_chain(st))
                return qcur, chains

            def emit_pv(pv, nkt, kts, n0s, ps):
                for h in pv:
                    for j, kt in enumerate(kts):
                        nc.tensor.matmul(
                            pv[h][:, n0s[j]:QW],
                            v_t[kt][:, h, :],
                            ps[h][:, j * QW + n0s[j]:(j + 1) * QW],
                            start=(kt == 0),
                            stop=(kt == nkt - 1),
                        )

            def attend_qt(qt, qcur, filler):
                """Attention for q tile qt; weaves `filler` chains into the
                PE stream. Returns attn tiles for the deferred o-proj."""
                nkt = 4 * qt + 4
                nkp = nkt // 2
                total_units = 4 * nkp
                stride = max(1, -(-total_units // max(1, len(filler)))) if filler else 10**9
                fi = iter(filler)
                unit = 0
                attn_t = [
                    pool_attn.tile([128, QW], MM_DT, tag=f"a{fc}", name=f"attn{fc}", bufs=2)
                    for fc in range(4)
                ]
                for hp in range(4):
                    heads = (2 * hp, 2 * hp + 1)
                    pv = {h: ps_pv.tile([65, QW], f32, tag="pv", name=f"pv{h}") for h in heads}
                    pending = None
                    for kp in range(nkp):
                        kts = (2 * kp, 2 * kp + 1)
                        n0s = [max(0, 128 * (kt - 4 * qt)) for kt in kts]
                        # score matmuls, both heads adjacent (disjoint PE row
                        # groups run concurrently)
                        scs = {}
                        for j, kt in enumerate(kts):
                            for h in heads:
                                if h not in scs:
                                    scs[h] = ps_score.tile(
                                        [128, 2 * QW], f32, tag="sc", name="sc"
                                    )
                                ht = h // 2
                                po = (h % 2) * 64
                                nc.tensor.matmul(
                                    scs[h][:, j * QW + n0s[j]:(j + 1) * QW],
                                    kT_t[ht][po:po + 64, kt * 128:(kt + 1) * 128],
                                    qcur[ht][po:po + 64, n0s[j]:QW],
                                    start=True,
                                    stop=True,
                                )
                        # exp for both heads; tri-mask on P afterwards (the
                        # masked PV is deferred one unit, so the DVE mask is
                        # off the critical path)
                        ps = {}
                        for h in heads:
                            ps[h] = pool_p.tile([128, 2 * QW], MM_DT, tag="p", name="p")
                            nc.scalar.activation(
                                ps[h][:, n0s[0]:2 * QW], scs[h][:, n0s[0]:2 * QW],
                                AF.Exp, scale=0.125,
                            )
                        for h in heads:
                            for j, kt in enumerate(kts):
                                if kt - 4 * qt >= 0:
                                    c0 = j * QW + n0s[j]
                                    nc.vector.tensor_tensor(
                                        ps[h][:, c0:c0 + 128],
                                        ps[h][:, c0:c0 + 128],
                                        tri_sb[:],
                                        mybir.AluOpType.mult,
                                    )
                        if pending is not None:
                            emit_pv(pv, nkt, *pending)
                        pending = (kts, n0s, ps)
                        if unit % stride == 0:
                            ch = next(fi, None)
                            if ch is not None:
                                ch()
                        unit += 1
                    emit_pv(pv, nkt, *pending)
                    for h in heads:
                        ht = h // 2
                        po = (h % 2) * 64
                        drow = pool_misc.tile([1, QW], f32, tag="drow", name="drow")
                        nc.vector.tensor_copy(drow[:], pv[h][64:65, :])
                        ocp = pool_misc.tile([64, QW], f32, tag="ocp", name="ocp", bufs=2)
                        nc.vector.tensor_copy(ocp[:], pv[h][0:64, :])
                        srow = pool_misc.tile([1, QW], f32, tag="srow", name="srow")
                        nc.vector.reciprocal_approx_accurate(
                            out=drow[:], in_=drow[:], scratch=srow[:]
                        )
                        dbc = pool_misc.tile([64, QW], f32, tag="dbc", name="dbc")
                        nc.gpsimd.partition_broadcast(dbc[:], drow[:])
                        nc.vector.tensor_tensor(
                            attn_t[ht][po:po + 64, :],
                            ocp[:],
                            dbc[:],
                            mybir.AluOpType.mult,
                        )
                # drain leftover filler
                for ch in fi:
                    ch()
                return attn_t

            def oproj_qt(qt, attn_t):
                for od in range(8):
                    yb = ps_a.tile([128, QW], f32, tag="qkv", name="yb")
                    for fc in range(4):
                        nc.tensor.matmul(
                            yb[:],
                            wo_t[fc][od // 4][:, (od % 4) * 128:(od % 4 + 1) * 128],
                            attn_t[fc][:],
                            start=(fc == 0),
                            stop=(fc == 3),
                        )
                    y_sb = pool_y.tile([128, QW], f32, tag="y", name="ysb")
                    nc.vector.tensor_copy(y_sb[:], yb[:])
                    nc.sync.dma_start(
                        yT_d[od * 128:(od + 1) * 128, qt * QW:(qt + 1) * QW], y_sb[:]
                    )

            # ---- token block 0 projections (inline) ----
            x0 = load_x(0)
            qcur, chains0 = make_proj_chains(0, x0)
            for ch in chains0:
                ch()

            # wo after the tb0 burst (first needed at attend(1) start)
            wo_t = []
            for fc in range(4):
                row = []
                for oh in range(2):
                    t = pool_w.tile([128, 512], MM_DT, tag=f"wo{fc}{oh}", name=f"wo{fc}{oh}")
                    nc.gpsimd.dma_start(
                        t[:], wo_d[fc * 128:(fc + 1) * 128, oh * 512:(oh + 1) * 512]
                    )
                    row.append(t)
                wo_t.append(row)

            prev_attn = None
            for qt in range(NQT):
                if qt + 1 < NTB:
                    x_next = load_x(qt + 1)
                    qcur_next, chains = make_proj_chains(qt + 1, x_next)
                else:
                    qcur_next, chains = None, []
                at = attend_qt(qt, qcur, chains)
                if prev_attn is not None:
                    oproj_qt(qt - 1, prev_attn)
                prev_attn = at
                qcur = qcur_next
            oproj_qt(NQT - 1, prev_attn)

    nc.compile()
    return nc


_NC_CACHE = None


def _ensure_trace_hooks():
    """Dev-only: register the axon NTFF profile hook if the image's antenv
    lacks it, and neuter the artifact upload (no bucket in this container)."""
    import sys
    import types

    import concourse.bass_utils as bu

    bu.upload_artifacts = lambda tmpdir: tmpdir
    try:
        from antenv.axon_hooks import get_axon_ntff_profile_hook  # noqa: F401
        return
    except ImportError:
        pass
    from trn_agent_boot.trn_boot import _ntff_profile_via_ctypes

    hook = _ntff_profile_via_ctypes("/opt/axon/libaxon_pjrt.so")
    mod = types.ModuleType("antenv.axon_hooks")
    mod.get_axon_ntff_profile_hook = lambda: hook

    def set_axon_ntff_profile_hook(h):
        mod.get_axon_ntff_profile_hook = lambda: h

    mod.set_axon_ntff_profile_hook = set_axon_ntff_profile_hook
    sys.modules["antenv.axon_hooks"] = mod


def kernel(x, q_proj, k_proj, v_proj, o_proj, trace=False):
    global _NC_CACHE
    if trace:
        _ensure_trace_hooks()
    x = np.asarray(x, dtype=np.float32)
    q_proj = np.asarray(q_proj, dtype=np.float32)
    k_proj = np.asarray(k_proj, dtype=np.float32)
    v_proj = np.asarray(v_proj, dtype=np.float32)
    o_proj = np.asarray(o_proj, dtype=np.float32)

    tri = np.zeros((128, 128), dtype=np.float32)
    k_idx = np.arange(128)[:, None]
    q_idx = np.arange(128)[None, :]
    tri[k_idx <= q_idx] = 1.0

    in_maps = []
    for b in range(B):
        xT = np.ascontiguousarray(x[b].T)
        for g in range(NHG):
            rows = slice(g * F, (g + 1) * F)
            in_maps.append(
                {
                    "xT": xT,
                    "wq": np.ascontiguousarray(q_proj[rows, :].T),
                    "wk": np.ascontiguousarray(k_proj[rows, :].T),
                    "wv": np.ascontiguousarray(v_proj[rows, :].T),
                    "wo": np.ascontiguousarray(o_proj[:, rows].T),
                    "tri": tri,
                }
            )

    if _NC_CACHE is None:
        _NC_CACHE = build_nc()
    nc = _NC_CACHE

    res = run_bass_kernel_spmd(nc, in_maps, list(range(2 * B)), trace=trace)

    y = np.empty((B, S, DM), dtype=np.float32)
    for b in range(B):
        acc = res.results[2 * b]["yT"] + res.results[2 * b + 1]["yT"]
        y[b] = acc.T
    if trace:
        return y, res
    return y
